# revision 43
# baseline (speedup 1.0000x reference)
"""Diagonal-Gaussian KL loss on 8 Trainium2 NeuronCores.

KL(p || q) summed over batch, with diag covariances exp(sigma):
  0.5 * [ sum(sigma_q - sigma_p) + sum(exp(sigma_p - sigma_q))
          + sum((mu_q-mu_p)^2 * exp(-sigma_q)) - B*D ]

Data-parallel over the batch dim: each core reduces a [1024, 2048] shard of
the four inputs; the tiny final combine happens on the host in float64.

The kernel is HBM-bound, so inputs are downcast host-side: sigmas to
float8_e3m4 (values are N(0,1), well inside e3m4 range; they only feed
exps and a linear sum, where the ~1% quantization noise cancels across
33M elements), mus to bfloat16 (keeps the mu-side elementwise ops on DVE
in its 2x 16-bit mode).  12KB/partition/tile instead of 32KB for f32.
2e-2 rel-err budget; measured result error ~1e-4.

Measured HW op costs ([128,2048] tiles): DVE 1-byte/stt ops 2.29us, DVE
bf16 tensor_tensor 1.22us (2x), ACT activation ~2.0us (+0.28 accum read),
PE ones-matmul ~0.7us per 512 cols, DMA 4.74us/tile.  GPSIMD's Q7 cores
do no compute - any Pool activity degrades concurrent DVE ops ~4x (SBUF
contention, measured directly) - but its cheap sequencer issues the mu
DMAs in parallel with Sync's sigma DMAs.

The u^2 reduction doesn't fit one engine at DMA pace, so it alternates:
ACT Square+accum on odd tiles, DVE u*u + PE ones-matmul sum on even.
  per row-tile i (8 per core), 3-slot DMA ring, 2-slot compute buffers:
    SYNC: big_s[i%3] <- sigma tile (fp8); GPSIMD: big_m[i%3] <- mu (bf16)
    DVE : stt  a = sp - sq, acc_a[i] = sum(a)   (fused sub+reduce)
          d = mq - mp
          u = d * e3
          [even] uu = u * u
    ACT : e3 = exp(-0.5*sq)
          exp(a) in-place, acc_t[i] = sum(.)    (accum_out)
          [odd] square(u), acc_m[i] = sum(u^2)
    PE  : [even] psum_sm[1,512] += ones^T @ uu chunks
  tail: ACT copies psum_sm to SBUF; SYNC DMAs acc + sm out.
"""

from contextlib import ExitStack

import ml_dtypes
import numpy as np

import concourse.bass as bass
from concourse import mybir
from concourse.bass_utils import run_bass_kernel_spmd

B, D = 8192, 2048
NCORES = 8
ROWS = B // NCORES  # rows per core
P = 128  # SBUF partitions
NT = ROWS // P  # row-tiles per core
NC = D // 512  # 512-col PSUM chunks per tile

F32 = mybir.dt.float32
BF16 = mybir.dt.bfloat16
F8E3 = mybir.dt.float8e3

A0, T0, M0 = 0, NT, 2 * NT  # acc column groups (M cols used only for ACT tiles)

ACT_SQ = [i for i in range(NT) if i % 2 == 1]  # squares via ACT Square+accum
DVE_SQ = [i for i in range(NT) if i % 2 == 0]  # squares via DVE mul + PE sum
PIDX = {i: n for n, i in enumerate(DVE_SQ)}  # PE-summed tile ordinal
MIDX = {i: n for n, i in enumerate(ACT_SQ)}  # ACT-square ordinal


def _build_nc():
    nc = bass.Bass(trn_type="TRN2", target_bir_lowering=False)

    xs = nc.dram_tensor("xs", [2, ROWS, D], F8E3, kind="ExternalInput")  # sq, sp
    xm = nc.dram_tensor("xm", [2, ROWS, D], BF16, kind="ExternalInput")  # mq, mp
    ones_in = nc.dram_tensor("ones_in", [P, 1], BF16, kind="ExternalInput")
    out = nc.dram_tensor("out", [P, 3 * NT], F32, kind="ExternalOutput")
    out2 = nc.dram_tensor("out2", [1, len(DVE_SQ) * 512], F32, kind="ExternalOutput")

    Exp = mybir.ActivationFunctionType.Exp
    Square = mybir.ActivationFunctionType.Square
    Alu = mybir.AluOpType

    ctx = ExitStack()
    with ctx:
        big_s = [ctx.enter_context(nc.sbuf_tensor(f"bs{k}", [P, 2 * D], F8E3)) for k in range(3)]
        big_m = [ctx.enter_context(nc.sbuf_tensor(f"bm{k}", [P, 2 * D], BF16)) for k in range(3)]
        a_b = [ctx.enter_context(nc.sbuf_tensor(f"a{j}", [P, D], BF16)) for j in range(2)]
        e3_b = [ctx.enter_context(nc.sbuf_tensor(f"e3{j}", [P, D], BF16)) for j in range(3)]
        u_b = [ctx.enter_context(nc.sbuf_tensor(f"u{j}", [P, D], BF16)) for j in range(2)]
        uu_b = [ctx.enter_context(nc.sbuf_tensor(f"uu{j}", [P, D], BF16)) for j in range(2)]
        d_b = ctx.enter_context(nc.sbuf_tensor("d", [P, D], BF16))
        ones = ctx.enter_context(nc.sbuf_tensor("ones", [P, 1], BF16))
        sm_sb = ctx.enter_context(nc.sbuf_tensor("sm_sb", [1, len(DVE_SQ) * 512], F32))
        acc = ctx.enter_context(nc.sbuf_tensor("acc", [P, 3 * NT], F32))
        sm_ps = ctx.enter_context(nc.psum_tensor("sm_ps", [1, len(DVE_SQ) * 512], F32))

        ds_s = [ctx.enter_context(nc.semaphore(f"dss{k}")) for k in range(3)]
        ds_m = [ctx.enter_context(nc.semaphore(f"dsm{k}")) for k in range(3)]
        cs = ctx.enter_context(nc.semaphore("cs"))
        v_sem = ctx.enter_context(nc.semaphore("v_sem"))  # DVE: stt_a, d, u per tile
        vu_sem = ctx.enter_context(nc.semaphore("vu_sem"))  # DVE uu (even tiles)
        a_sem = ctx.enter_context(nc.semaphore("a_sem"))  # ACT: e3, exp_acc (+final copy)
        m_sem = ctx.enter_context(nc.semaphore("m_sem"))  # ACT square_acc (odd tiles)
        pe_sem = ctx.enter_context(nc.semaphore("pe_sem"))  # PE sum per even tile
        out_sem = ctx.enter_context(nc.semaphore("out_sem"))

        # DRAM APs for row-tile i: partitions = rows r..r+127, free = (t, d).
        def s_tile_ap(i):
            return bass.AP(xs, i * P * D, [[D, P], [ROWS * D, 2], [1, D]])

        def m_tile_ap(i):
            return bass.AP(xm, i * P * D, [[D, P], [ROWS * D, 2], [1, D]])

        with nc.Block(no_gpsimd_drain=True) as block:

            @block.sync
            def _(sync):
                for i in range(NT):
                    k = i % 3
                    if i >= 3:
                        # sigma slot released by its readers stt_a and e3
                        sync.wait_ge(v_sem, 3 * (i - 3) + 1)
                        sync.wait_ge(a_sem, 2 * (i - 3) + 1)
                    sync.dma_start(big_s[k][:, :], s_tile_ap(i)).then_inc(ds_s[k], 16)
                sync.wait_ge(a_sem, 2 * NT + 1)  # psum->sbuf copy retired
                sync.dma_start(out[:, :], acc[:, :]).then_inc(out_sem, 16)
                sync.dma_start(out2[:, :], sm_sb[:, :]).then_inc(out_sem, 16)
                sync.wait_ge(out_sem, 32)  # both output DMAs done

            @block.vector
            def _(vector):
                for i in range(NT):
                    k, j = i % 3, i % 2
                    vector.wait_ge(ds_s[k], 16 * (i // 3 + 1))  # sigma tile landed
                    if i >= 2:
                        vector.wait_ge(a_sem, 2 * (i - 2) + 2)  # a[j] freed by exp_acc(i-2)
                    sq_t = big_s[k][:, 0:D]
                    sp_t = big_s[k][:, D : 2 * D]
                    vector.scalar_tensor_tensor(
                        a_b[j][:, :], sp_t, 0.0, sq_t, Alu.bypass, Alu.subtract,
                        accum_out=acc[:, A0 + i : A0 + i + 1],
                    ).then_inc(v_sem, 1)
                    vector.wait_ge(ds_m[k], 16 * (i // 3 + 1))  # mu tile landed
                    vector.tensor_sub(
                        d_b[:, :], big_m[k][:, 0:D], big_m[k][:, D : 2 * D]
                    ).then_inc(v_sem, 1)
                    vector.wait_ge(a_sem, 2 * i + 1)  # e3(i) ready
                    if i >= 2 and (i - 2) in MIDX:
                        vector.wait_ge(m_sem, MIDX[i - 2] + 1)  # u[j] freed by ACT sq
                    vector.tensor_mul(
                        u_b[j][:, :], d_b[:, :], e3_b[i % 3][:, :]
                    ).then_inc(v_sem, 1)
                    if i in PIDX:
                        # uu[jj] freed by PE sum two even-tiles back
                        if PIDX[i] >= 2:
                            vector.wait_ge(pe_sem, PIDX[i] - 1)
                        vector.tensor_mul(
                            uu_b[PIDX[i] % 2][:, :], u_b[j][:, :], u_b[j][:, :]
                        ).then_inc(vu_sem, 1)

            @block.scalar
            def _(scalar):
                scalar.dma_start(ones[:, :], ones_in[:, :]).then_inc(cs, 16)
                for i in range(NT):
                    k, j = i % 3, i % 2
                    scalar.wait_ge(ds_s[k], 16 * (i // 3 + 1))  # sigma tile landed
                    if i >= 3:
                        scalar.wait_ge(v_sem, 3 * (i - 3) + 3)  # e3[i%3] freed by u(i-3)
                    scalar.activation(
                        e3_b[i % 3][:, :], big_s[k][:, 0:D], Exp, scale=-0.5
                    ).then_inc(a_sem, 1)
                    scalar.wait_ge(v_sem, 3 * i + 1)  # a(i) ready
                    scalar.activation(
                        a_b[j][:, :], a_b[j][:, :], Exp,
                        accum_out=acc[:, T0 + i : T0 + i + 1],
                    ).then_inc(a_sem, 1)
                    if i in MIDX:
                        scalar.wait_ge(v_sem, 3 * i + 3)  # u(i) ready
                        scalar.activation(
                            u_b[j][:, :], u_b[j][:, :], Square,
                            accum_out=acc[:, M0 + i : M0 + i + 1],
                        ).then_inc(m_sem, 1)
                scalar.wait_ge(pe_sem, len(DVE_SQ))  # all u^2 sums accumulated
                scalar.copy(sm_sb[:, :], sm_ps[:, :]).then_inc(a_sem, 1)

            @block.gpsimd
            def _(gpsimd):
                # GPSIMD only issues the mu DMAs (25ns sequencer cost, its
                # Q7 cores stay idle - Q7 compute would poison DVE throughput).
                # mu slot k is free once DVE's d(i-3) consumed it.
                for i in range(NT):
                    k = i % 3
                    if i >= 3:
                        gpsimd.wait_ge(v_sem, 3 * (i - 3) + 2)  # d(i-3) done
                    gpsimd.dma_start(big_m[k][:, :], m_tile_ap(i)).then_inc(ds_m[k], 16)

            @block.tensor
            def _(pe):
                pe.wait_ge(cs, 16)  # ones loaded via DMA
                # warm-up matmul: absorbs the window between the ones DMA
                # completion signal and its SBUF write becoming visible to
                # PE weight loads (first-execution NaN otherwise); the [1,1]
                # result lands in a region reset by the first real group.
                pe.matmul(sm_ps[:, 0:1], ones[:, :], ones[:, 0:1],
                          start=True, stop=True)
                for n, i in enumerate(DVE_SQ):
                    jj = n % 2
                    pe.wait_ge(vu_sem, n + 1)  # uu ready
                    for c in range(NC):
                        # per-tile accumulation group in its own PSUM bank
                        mm = pe.matmul(
                            sm_ps[:, n * 512 : (n + 1) * 512], ones[:, :],
                            uu_b[jj][:, c * 512 : (c + 1) * 512],
                            start=(c == 0),
                            stop=(c == NC - 1),
                        )
                    mm.then_inc(pe_sem, 1)

    return nc


_NC = None


def _get_nc():
    global _NC
    if _NC is None:
        _NC = _build_nc()
    return _NC


def _run(inputs, **kw):
    sig = np.stack(
        [
            np.asarray(inputs["sigma_q"], dtype=np.float32),
            np.asarray(inputs["sigma_p"], dtype=np.float32),
        ],
        axis=0,
    ).astype(ml_dtypes.float8_e3m4)  # [2, B, D]
    mus = np.stack(
        [
            np.asarray(inputs["mu_q"], dtype=np.float32),
            np.asarray(inputs["mu_p"], dtype=np.float32),
        ],
        axis=0,
    ).astype(ml_dtypes.bfloat16)  # [2, B, D]
    ones_v = np.ones((P, 1), dtype=np.float32).astype(ml_dtypes.bfloat16)
    in_maps = [
        {
            "xs": np.ascontiguousarray(sig[:, c * ROWS : (c + 1) * ROWS, :]),
            "xm": np.ascontiguousarray(mus[:, c * ROWS : (c + 1) * ROWS, :]),
            "ones_in": ones_v,
        }
        for c in range(NCORES)
    ]
    return run_bass_kernel_spmd(_get_nc(), in_maps, core_ids=list(range(NCORES)), **kw)


def _combine(results):
    # partial sums -> scalar, in f64 for a clean final reduction
    S = np.stack([r["out"] for r in results]).astype(np.float64)
    M = np.stack([r["out2"] for r in results]).astype(np.float64)
    s_a = S[..., A0:T0].sum()  # sum(sigma_p - sigma_q)
    s_t = S[..., T0:M0].sum()  # sum(exp(sigma_p - sigma_q))
    # maha: PE-summed even tiles + ACT-accumulated odd tiles
    s_m = M.sum() + S[..., [M0 + i for i in ACT_SQ]].sum()
    kl = 0.5 * (-s_a + s_t + s_m - B * D)
    return np.asarray(kl, dtype=np.float32)


def kernel(**inputs):
    return _combine(_run(inputs).results)


def run_traced(inputs, **kw):
    """test.py helper: returns (value, BassKernelResults) with profiling."""
    br = _run(inputs, trace=True, **kw)
    return _combine(br.results), br


# revision 44
# speedup vs baseline: 1.0313x; 1.0313x over previous
"""Diagonal-Gaussian KL loss on 8 Trainium2 NeuronCores.

KL(p || q) summed over batch, with diag covariances exp(sigma):
  0.5 * [ sum(sigma_q - sigma_p) + sum(exp(sigma_p - sigma_q))
          + sum((mu_q-mu_p)^2 * exp(-sigma_q)) - B*D ]

Data-parallel over the batch dim: each core reduces a [1024, 2048] shard of
the four inputs; the tiny final combine happens on the host in float64.

The kernel is HBM-bound, so inputs are downcast host-side: sigmas to
float8_e3m4 (values are N(0,1), well inside e3m4 range; they only feed
exps and a linear sum, where the ~1% quantization noise cancels across
33M elements), mus to bfloat16 (keeps the mu-side elementwise ops on DVE
in its 2x 16-bit mode).  12KB/partition/tile instead of 32KB for f32.
2e-2 rel-err budget; measured result error ~1e-4.

Measured HW op costs ([128,2048] tiles): DVE 1-byte/stt ops 2.29us, DVE
bf16 tensor_tensor 1.22us (2x), ACT activation ~2.0us (+0.28 accum read),
PE ones-matmul ~0.7us per 512 cols, DMA 4.74us/tile.  GPSIMD's Q7 cores
do no compute - any Pool activity degrades concurrent DVE ops ~4x (SBUF
contention, measured directly) - but its cheap sequencer issues the mu
DMAs in parallel with Sync's sigma DMAs.

The u^2 reduction doesn't fit one engine at DMA pace, so it alternates:
ACT Square+accum on odd tiles, DVE u*u + PE ones-matmul sum on even.
  per row-tile i (8 per core), 3-slot DMA ring, 2-slot compute buffers:
    SYNC: big_s[i%3] <- sigma tile (fp8); GPSIMD: big_m[i%3] <- mu (bf16)
    DVE : stt  a = sp - sq, acc_a[i] = sum(a)   (fused sub+reduce)
          d = mq - mp
          u = d * e3
          [even] uu = u * u
    ACT : e3 = exp(-0.5*sq)
          exp(a) in-place, acc_t[i] = sum(.)    (accum_out)
          [odd] square(u), acc_m[i] = sum(u^2)
    PE  : [even] psum_sm[1,512] += ones^T @ uu chunks
  tail: ACT copies psum_sm to SBUF; SYNC DMAs acc + sm out.
"""

from contextlib import ExitStack

import ml_dtypes
import numpy as np

import concourse.bass as bass
from concourse import mybir
from concourse.bass_utils import run_bass_kernel_spmd

B, D = 8192, 2048
NCORES = 8
ROWS = B // NCORES  # rows per core
P = 128  # SBUF partitions
NT = ROWS // P  # row-tiles per core
NC = D // 512  # 512-col PSUM chunks per tile

F32 = mybir.dt.float32
BF16 = mybir.dt.bfloat16
F8E3 = mybir.dt.float8e3

A0, T0, M0 = 0, NT, 2 * NT  # acc column groups (M cols used only for ACT tiles)

ACT_SQ = [i for i in range(NT) if i % 2 == 1]  # squares via ACT Square+accum
DVE_SQ = [i for i in range(NT) if i % 2 == 0]  # squares via DVE mul + PE sum
PIDX = {i: n for n, i in enumerate(DVE_SQ)}  # PE-summed tile ordinal
MIDX = {i: n for n, i in enumerate(ACT_SQ)}  # ACT-square ordinal


def _build_nc():
    nc = bass.Bass(trn_type="TRN2", target_bir_lowering=False)

    xs = nc.dram_tensor("xs", [2, ROWS, D], F8E3, kind="ExternalInput")  # sq, sp
    xm = nc.dram_tensor("xm", [2, ROWS, D], BF16, kind="ExternalInput")  # mq, mp
    ones_in = nc.dram_tensor("ones_in", [P, 1], BF16, kind="ExternalInput")
    out = nc.dram_tensor("out", [P, 3 * NT], F32, kind="ExternalOutput")
    out2 = nc.dram_tensor("out2", [1, len(DVE_SQ) * 512], F32, kind="ExternalOutput")

    Exp = mybir.ActivationFunctionType.Exp
    Square = mybir.ActivationFunctionType.Square
    Alu = mybir.AluOpType

    ctx = ExitStack()
    with ctx:
        big_s = [ctx.enter_context(nc.sbuf_tensor(f"bs{k}", [P, 2 * D], F8E3)) for k in range(3)]
        big_m = [ctx.enter_context(nc.sbuf_tensor(f"bm{k}", [P, 2 * D], BF16)) for k in range(3)]
        a_b = [ctx.enter_context(nc.sbuf_tensor(f"a{j}", [P, D], BF16)) for j in range(2)]
        e3_b = [ctx.enter_context(nc.sbuf_tensor(f"e3{j}", [P, D], BF16)) for j in range(2)]
        u_b = [ctx.enter_context(nc.sbuf_tensor(f"u{j}", [P, D], BF16)) for j in range(2)]
        uu_b = [ctx.enter_context(nc.sbuf_tensor(f"uu{j}", [P, D], BF16)) for j in range(2)]
        d_b = ctx.enter_context(nc.sbuf_tensor("d", [P, D], BF16))
        ones = ctx.enter_context(nc.sbuf_tensor("ones", [P, 1], BF16))
        sm_sb = ctx.enter_context(nc.sbuf_tensor("sm_sb", [1, len(DVE_SQ) * 512], F32))
        acc = ctx.enter_context(nc.sbuf_tensor("acc", [P, 3 * NT], F32))
        sm_ps = ctx.enter_context(nc.psum_tensor("sm_ps", [1, len(DVE_SQ) * 512], F32))

        ds_s = [ctx.enter_context(nc.semaphore(f"dss{k}")) for k in range(3)]
        ds_m = [ctx.enter_context(nc.semaphore(f"dsm{k}")) for k in range(3)]
        cs = ctx.enter_context(nc.semaphore("cs"))
        v_sem = ctx.enter_context(nc.semaphore("v_sem"))  # DVE: stt_a, d, u per tile
        vu_sem = ctx.enter_context(nc.semaphore("vu_sem"))  # DVE uu (even tiles)
        a_sem = ctx.enter_context(nc.semaphore("a_sem"))  # ACT: e3, exp_acc (+final copy)
        m_sem = ctx.enter_context(nc.semaphore("m_sem"))  # ACT square_acc (odd tiles)
        pe_sem = ctx.enter_context(nc.semaphore("pe_sem"))  # PE sum per even tile
        out_sem = ctx.enter_context(nc.semaphore("out_sem"))

        # DRAM APs for row-tile i: partitions = rows r..r+127, free = (t, d).
        def s_tile_ap(i):
            return bass.AP(xs, i * P * D, [[D, P], [ROWS * D, 2], [1, D]])

        def m_tile_ap(i):
            return bass.AP(xm, i * P * D, [[D, P], [ROWS * D, 2], [1, D]])

        with nc.Block(no_gpsimd_drain=True) as block:

            @block.sync
            def _(sync):
                for i in range(NT):
                    k = i % 3
                    if i >= 3:
                        # sigma slot released by its readers stt_a and e3
                        sync.wait_ge(v_sem, 3 * (i - 3) + 1)
                        sync.wait_ge(a_sem, 2 * (i - 3) + 1)
                    sync.dma_start(big_s[k][:, :], s_tile_ap(i)).then_inc(ds_s[k], 16)
                sync.wait_ge(a_sem, 2 * NT + 1)  # psum->sbuf copy retired
                sync.dma_start(out[:, :], acc[:, :]).then_inc(out_sem, 16)
                sync.dma_start(out2[:, :], sm_sb[:, :]).then_inc(out_sem, 16)
                sync.wait_ge(out_sem, 32)  # both output DMAs done

            @block.vector
            def _(vector):
                for i in range(NT):
                    k, j = i % 3, i % 2
                    vector.wait_ge(ds_s[k], 16 * (i // 3 + 1))  # sigma tile landed
                    if i >= 2:
                        vector.wait_ge(a_sem, 2 * (i - 2) + 2)  # a[j] freed by exp_acc(i-2)
                    sq_t = big_s[k][:, 0:D]
                    sp_t = big_s[k][:, D : 2 * D]
                    vector.scalar_tensor_tensor(
                        a_b[j][:, :], sp_t, 0.0, sq_t, Alu.bypass, Alu.subtract,
                        accum_out=acc[:, A0 + i : A0 + i + 1],
                    ).then_inc(v_sem, 1)
                    vector.wait_ge(ds_m[k], 16 * (i // 3 + 1))  # mu tile landed
                    vector.tensor_sub(
                        d_b[:, :], big_m[k][:, 0:D], big_m[k][:, D : 2 * D]
                    ).then_inc(v_sem, 1)
                    vector.wait_ge(a_sem, 2 * i + 1)  # e3(i) ready
                    if i >= 2 and (i - 2) in MIDX:
                        vector.wait_ge(m_sem, MIDX[i - 2] + 1)  # u[j] freed by ACT sq
                    vector.tensor_mul(
                        u_b[j][:, :], d_b[:, :], e3_b[j][:, :]
                    ).then_inc(v_sem, 1)
                    if i in PIDX:
                        # uu[jj] freed by PE sum two even-tiles back
                        if PIDX[i] >= 2:
                            vector.wait_ge(pe_sem, PIDX[i] - 1)
                        vector.tensor_mul(
                            uu_b[PIDX[i] % 2][:, :], u_b[j][:, :], u_b[j][:, :]
                        ).then_inc(vu_sem, 1)

            @block.scalar
            def _(scalar):
                scalar.dma_start(ones[:, :], ones_in[:, :]).then_inc(cs, 16)
                for i in range(NT):
                    k, j = i % 3, i % 2
                    scalar.wait_ge(ds_s[k], 16 * (i // 3 + 1))  # sigma tile landed
                    if i >= 2:
                        scalar.wait_ge(v_sem, 3 * (i - 2) + 3)  # e3[j] freed by u(i-2)
                    scalar.activation(
                        e3_b[j][:, :], big_s[k][:, 0:D], Exp, scale=-0.5
                    ).then_inc(a_sem, 1)
                    scalar.wait_ge(v_sem, 3 * i + 1)  # a(i) ready
                    scalar.activation(
                        a_b[j][:, :], a_b[j][:, :], Exp,
                        accum_out=acc[:, T0 + i : T0 + i + 1],
                    ).then_inc(a_sem, 1)
                    if i in MIDX:
                        scalar.wait_ge(v_sem, 3 * i + 3)  # u(i) ready
                        scalar.activation(
                            u_b[j][:, :], u_b[j][:, :], Square,
                            accum_out=acc[:, M0 + i : M0 + i + 1],
                        ).then_inc(m_sem, 1)
                scalar.wait_ge(pe_sem, len(DVE_SQ))  # all u^2 sums accumulated
                scalar.copy(sm_sb[:, :], sm_ps[:, :]).then_inc(a_sem, 1)

            @block.gpsimd
            def _(gpsimd):
                # GPSIMD only issues the mu DMAs (25ns sequencer cost, its
                # Q7 cores stay idle - Q7 compute would poison DVE throughput).
                # mu slot k is free once DVE's d(i-3) consumed it.
                for i in range(NT):
                    k = i % 3
                    if i >= 3:
                        gpsimd.wait_ge(v_sem, 3 * (i - 3) + 2)  # d(i-3) done
                    gpsimd.dma_start(big_m[k][:, :], m_tile_ap(i)).then_inc(ds_m[k], 16)

            @block.tensor
            def _(pe):
                pe.wait_ge(cs, 16)  # ones loaded via DMA
                # warm-up matmul: absorbs the window between the ones DMA
                # completion signal and its SBUF write becoming visible to
                # PE weight loads (first-execution NaN otherwise); the [1,1]
                # result lands in a region reset by the first real group.
                pe.matmul(sm_ps[:, 0:1], ones[:, :], ones[:, 0:1],
                          start=True, stop=True)
                for n, i in enumerate(DVE_SQ):
                    jj = n % 2
                    pe.wait_ge(vu_sem, n + 1)  # uu ready
                    for c in range(NC):
                        # per-tile accumulation group in its own PSUM bank
                        mm = pe.matmul(
                            sm_ps[:, n * 512 : (n + 1) * 512], ones[:, :],
                            uu_b[jj][:, c * 512 : (c + 1) * 512],
                            start=(c == 0),
                            stop=(c == NC - 1),
                        )
                    mm.then_inc(pe_sem, 1)

    return nc


_NC = None


def _get_nc():
    global _NC
    if _NC is None:
        _NC = _build_nc()
    return _NC


def _run(inputs, **kw):
    sig = np.stack(
        [
            np.asarray(inputs["sigma_q"], dtype=np.float32),
            np.asarray(inputs["sigma_p"], dtype=np.float32),
        ],
        axis=0,
    ).astype(ml_dtypes.float8_e3m4)  # [2, B, D]
    mus = np.stack(
        [
            np.asarray(inputs["mu_q"], dtype=np.float32),
            np.asarray(inputs["mu_p"], dtype=np.float32),
        ],
        axis=0,
    ).astype(ml_dtypes.bfloat16)  # [2, B, D]
    ones_v = np.ones((P, 1), dtype=np.float32).astype(ml_dtypes.bfloat16)
    in_maps = [
        {
            "xs": np.ascontiguousarray(sig[:, c * ROWS : (c + 1) * ROWS, :]),
            "xm": np.ascontiguousarray(mus[:, c * ROWS : (c + 1) * ROWS, :]),
            "ones_in": ones_v,
        }
        for c in range(NCORES)
    ]
    return run_bass_kernel_spmd(_get_nc(), in_maps, core_ids=list(range(NCORES)), **kw)


def _combine(results):
    # partial sums -> scalar, in f64 for a clean final reduction
    S = np.stack([r["out"] for r in results]).astype(np.float64)
    M = np.stack([r["out2"] for r in results]).astype(np.float64)
    s_a = S[..., A0:T0].sum()  # sum(sigma_p - sigma_q)
    s_t = S[..., T0:M0].sum()  # sum(exp(sigma_p - sigma_q))
    # maha: PE-summed even tiles + ACT-accumulated odd tiles
    s_m = M.sum() + S[..., [M0 + i for i in ACT_SQ]].sum()
    kl = 0.5 * (-s_a + s_t + s_m - B * D)
    return np.asarray(kl, dtype=np.float32)


def kernel(**inputs):
    return _combine(_run(inputs).results)


def run_traced(inputs, **kw):
    """test.py helper: returns (value, BassKernelResults) with profiling."""
    br = _run(inputs, trace=True, **kw)
    return _combine(br.results), br
